# revision 4
# baseline (speedup 1.0000x reference)
"""Trainium2 Bass kernel for nn_LutLinear (BCQ/LUT-quantized linear layer).

Math (K=4096, N=4096, WBIT=3, GROUP=128, APOT=3):
  bits[k, b, n]  = bit (k%32) of binaryWeight[k//32, b, n]
  B              = 2*bits - 1                        (in {-1, +1})
  scale[n, b, g] = sum_a 2^alpha[n, b, g, a]
  out[n] = sum_{g,b} scale[n,b,g] * (sum_{k in group g} x[k] * B[k,b,n]) + bias[n]

Strategy (tensor-parallel over N, 8 cores, N'=512 each):
  * Bit-unpack on DVE: one int32 tensor_scalar (shift + AND 0x40404040) per
    bit-in-byte position s extracts FOUR bit-planes at once -- the masked
    int32, bitcast to fp8e4m3, holds value 2.0*bit in each of its 4 bytes.
  * PE computes partial[g,b,n'] = 2*sum_{k in g} x_k*bit via 96 accumulating
    matmuls: lhsT = block-diagonal x matrix [128, 32] (bf16), rhs = strided
    fp8 bit-plane view [128, n-chunk].  The {-1,+1} offset is fixed up by
    subtracting S_g = sum_{k in g} x_k (per-partition scalar) afterwards.
  * scale from alpha via ACT exp(ln2 * alpha); final contraction over (g,b)
    as PE transpose + DVE multiply-reduce; add bias; DMA out.
"""

import os
import sys

for _p in ("/opt/trn_rl_repo", "/opt/pypackages"):
    if os.path.isdir(_p) and _p not in sys.path:
        sys.path.insert(0, _p)

from contextlib import ExitStack

import ml_dtypes
import numpy as np

import concourse.bass as bass
import concourse.tile as tile
from concourse import bacc, mybir
from concourse._compat import with_exitstack
from concourse.bass_utils import run_bass_kernel_spmd

K = 4096
N = 4096
GROUP = 128
WBIT = 3
NUM_APOT = 3
G = K // GROUP          # 32 groups
NCORES = 8
NS = N // NCORES        # 512 output features per core
NCHUNK = 2              # n' chunks per core for DMA/compute pipelining
CH = NS // NCHUNK       # 256
NBLK = NS // 128        # 4 partition-blocks of n' in the tail
WORDS = K // 32         # 128 packed words per (b, n)
LN2 = float(np.log(2.0))

_CACHE = {}


@with_exitstack
def _build_kernel_body(ctx: ExitStack, tc):
    nc = tc.nc
    f32 = mybir.dt.float32
    i32 = mybir.dt.int32
    bf16 = mybir.dt.bfloat16
    f8 = mybir.dt.float8e4

    bw = nc.dram_tensor("bw", [WORDS, WBIT * NS], i32, kind="ExternalInput")
    xall = nc.dram_tensor("xall", [WORDS, G * G], bf16, kind="ExternalInput")
    alphaf = nc.dram_tensor(
        "alphaf", [128, NBLK * WBIT * G * NUM_APOT], f32, kind="ExternalInput"
    )
    sg96 = nc.dram_tensor("sg96", [WBIT * G, 1], f32, kind="ExternalInput")
    biasb = nc.dram_tensor("biasb", [128, NBLK], f32, kind="ExternalInput")
    ident = nc.dram_tensor("ident", [WBIT * G, WBIT * G], f32, kind="ExternalInput")
    out = nc.dram_tensor("out", [128, NBLK], f32, kind="ExternalOutput")

    const = ctx.enter_context(tc.tile_pool(name="const", bufs=1))
    wpool = ctx.enter_context(tc.tile_pool(name="wpool", bufs=2))
    ppool = ctx.enter_context(tc.tile_pool(name="ppool", bufs=2))
    tailp = ctx.enter_context(tc.tile_pool(name="tailp", bufs=1))
    psum = ctx.enter_context(tc.tile_pool(name="psum", bufs=1, space="PSUM"))
    psum_t = ctx.enter_context(tc.tile_pool(name="psum_t", bufs=1, space="PSUM"))

    # --- small constant loads -------------------------------------------------
    xall_sb = const.tile([WORDS, G * G], bf16)
    nc.sync.dma_start(xall_sb[:], xall[:, :])
    sg_sb = const.tile([WBIT * G, 1], f32)
    nc.sync.dma_start(sg_sb[:], sg96[:, :])
    bias_sb = const.tile([128, NBLK], f32)
    nc.sync.dma_start(bias_sb[:], biasb[:, :])
    id_sb = const.tile([WBIT * G, WBIT * G], f32)
    nc.sync.dma_start(id_sb[:], ident[:, :])
    al_sb = const.tile([128, NBLK * WBIT * G * NUM_APOT], f32)
    nc.sync.dma_start(al_sb[:], alphaf[:, :])

    # --- scale[n', (blk, b, g)] = sum_a 2^alpha -------------------------------
    QA = NBLK * WBIT * G  # 384
    scf = tailp.tile([128, QA * NUM_APOT], f32)
    nc.scalar.activation(scf[:], al_sb[:], mybir.ActivationFunctionType.Exp, scale=LN2)
    scf3 = scf[:].rearrange("p (q a) -> p q a", a=NUM_APOT)
    sctmp = tailp.tile([128, QA], f32)
    nc.vector.tensor_tensor(
        sctmp[:], scf3[:, :, 0], scf3[:, :, 1], mybir.AluOpType.add
    )
    scale_t = tailp.tile([128, QA], f32)
    nc.vector.tensor_tensor(
        scale_t[:], sctmp[:], scf3[:, :, 2], mybir.AluOpType.add
    )

    # --- main loop: unpack bit-planes + accumulate matmuls --------------------
    # psum96[b*32+g, n'] accumulates 2 * sum_{k in g} x_k * bit[k, b, n']
    psum96 = psum.tile([WBIT * G, NS], f32)
    for ch in range(NCHUNK):
        wsb = wpool.tile([WORDS, WBIT * CH], i32, tag="wsb")
        bw3 = bw[:, :].rearrange("p (b n) -> p b n", b=WBIT)
        nc.sync.dma_start(wsb[:], bw3[:, :, ch * CH : (ch + 1) * CH])

        planes = []
        for s in range(8):
            ps = ppool.tile([WORDS, WBIT * CH], i32, tag=f"ps{s}")
            if s < 7:
                nc.vector.tensor_scalar(
                    ps[:],
                    wsb[:],
                    6 - s,
                    0x40404040,
                    mybir.AluOpType.logical_shift_left,
                    mybir.AluOpType.bitwise_and,
                )
            else:
                nc.vector.tensor_scalar(
                    ps[:],
                    wsb[:],
                    1,
                    0x40404040,
                    mybir.AluOpType.logical_shift_right,
                    mybir.AluOpType.bitwise_and,
                )
            planes.append(ps[:].bitcast(f8))  # [128, 4*WBIT*CH]

        for b in range(WBIT):
            acc = psum96[32 * b : 32 * b + 32, ch * CH : (ch + 1) * CH]
            for j in range(32):
                c, s = j // 8, j % 8
                base = 4 * (b * CH) + c
                rhs = planes[s][:, base : base + 4 * (CH - 1) + 1 : 4]  # [128,CH] fp8
                nc.tensor.matmul(
                    acc,
                    xall_sb[:, j * G : (j + 1) * G],  # [128, 32] bf16
                    rhs,
                    start=(j == 0),
                    stop=(j == 31),
                )

    # --- tail -----------------------------------------------------------------
    # part96 = psum96 - S_g  (per-partition scalar), PSUM -> SBUF
    part96 = tailp.tile([WBIT * G, NS], f32)
    nc.vector.tensor_scalar(
        part96[:], psum96[:], sg_sb[:], None, mybir.AluOpType.subtract
    )

    prod = tailp.tile([128, NBLK * WBIT * G], f32)
    for blk in range(NBLK):
        pt = psum_t.tile([128, WBIT * G], f32, tag=f"pt{blk}")
        nc.tensor.transpose(
            pt[:], part96[:, blk * 128 : (blk + 1) * 128], id_sb[:]
        )
        nc.vector.tensor_tensor(
            prod[:, blk * 96 : (blk + 1) * 96],
            pt[:],
            scale_t[:, blk * 96 : (blk + 1) * 96],
            mybir.AluOpType.mult,
        )

    red = tailp.tile([128, NBLK], f32)
    nc.vector.tensor_reduce(
        red[:],
        prod[:].rearrange("p (blk q) -> p blk q", blk=NBLK),
        mybir.AxisListType.X,
        mybir.AluOpType.add,
    )
    out_sb = tailp.tile([128, NBLK], f32)
    nc.vector.tensor_tensor(out_sb[:], red[:], bias_sb[:], mybir.AluOpType.add)
    nc.sync.dma_start(out[:, :], out_sb[:])


def _get_nc():
    if "nc" not in _CACHE:
        nc = bacc.Bacc(
            "TRN2",
            target_bir_lowering=False,
            debug=False,
            enable_asserts=False,
            num_devices=NCORES,
        )
        with tile.TileContext(nc) as tc:
            _build_kernel_body(tc)
        nc.compile()
        _CACHE["nc"] = nc
    return _CACHE["nc"]


def _prep_inputs(x, binaryWeight, alpha, bias):
    """Host-side shard + layout prep (no arithmetic beyond tiny x-side sums)."""
    x = np.asarray(x, dtype=np.float32).reshape(K)
    binaryWeight = np.asarray(binaryWeight, dtype=np.int32)
    alpha = np.asarray(alpha, dtype=np.int32)
    bias = np.asarray(bias, dtype=np.float32).reshape(N)

    # Block-diagonal lhsT bank: xall[w, j*32 + g] = x[128g + 32(w-4g) + j]
    xall = np.zeros((WORDS, G, G), dtype=np.float32)  # [w, j, g]
    k = np.arange(K)
    g = k // GROUP
    sub = (k % GROUP) // 32
    j = k % 32
    xall[4 * g + sub, j, g] = x
    xall = xall.reshape(WORDS, G * G).astype(ml_dtypes.bfloat16)

    # S_g per group, tiled over b -> [96, 1]
    sg = x.reshape(G, GROUP).sum(axis=1).astype(np.float32)
    sg96 = np.tile(sg, WBIT).reshape(WBIT * G, 1)

    ident = np.eye(WBIT * G, dtype=np.float32)

    in_maps = []
    for cc in range(NCORES):
        nsl = slice(cc * NS, (cc + 1) * NS)
        bw_sh = np.ascontiguousarray(binaryWeight[:, :, nsl]).reshape(
            WORDS, WBIT * NS
        )
        # alpha[n', b, g, a] -> [p, blk, b, g, a]  (n' = blk*128 + p)
        al = alpha[nsl].reshape(NBLK, 128, WBIT, G, NUM_APOT)
        al = np.ascontiguousarray(al.transpose(1, 0, 2, 3, 4)).astype(np.float32)
        al = al.reshape(128, NBLK * WBIT * G * NUM_APOT)
        bi = np.ascontiguousarray(
            bias[nsl].reshape(NBLK, 128).T
        )  # [p, blk]
        in_maps.append(
            {
                "bw": bw_sh,
                "xall": xall,
                "alphaf": al,
                "sg96": sg96,
                "biasb": bi,
                "ident": ident,
            }
        )
    return in_maps


def _run(inputs, trace=False, **kw):
    nc = _get_nc()
    in_maps = _prep_inputs(**inputs)
    res = run_bass_kernel_spmd(
        nc, in_maps, core_ids=list(range(NCORES)), trace=trace, **kw
    )
    outs = []
    for cc in range(NCORES):
        o = res.results[cc]["out"]  # [128, NBLK]
        outs.append(np.ascontiguousarray(o.T).reshape(NS))  # n' = blk*128 + p
    full = np.concatenate(outs).reshape(1, N).astype(np.float32)
    return full, res


def kernel(**inputs):
    out, _ = _run(inputs, trace=False)
    return out


# revision 5
# speedup vs baseline: 1.6113x; 1.6113x over previous
"""Trainium2 Bass kernel for nn_LutLinear (BCQ/LUT-quantized linear layer).

Math (K=4096, N=4096, WBIT=3, GROUP=128, APOT=3):
  bits[k, b, n]  = bit (k%32) of binaryWeight[k//32, b, n]
  B              = 2*bits - 1                        (in {-1, +1})
  scale[n, b, g] = sum_a 2^alpha[n, b, g, a]
  out[n] = sum_{g,b} scale[n,b,g] * (sum_{k in group g} x[k] * B[k,b,n]) + bias[n]

Strategy (tensor-parallel over N, 8 cores, N'=512 each):
  * Bit-unpack on DVE: one int32 tensor_scalar (shift + AND 0x40404040) per
    bit-in-byte position s extracts FOUR bit-planes at once -- the masked
    int32, bitcast to fp8e4m3, holds value 2.0*bit in each of its 4 bytes.
  * PE computes partial[g,b,n'] = 2*sum_{k in g} x_k*bit via 96 accumulating
    matmuls: lhsT = block-diagonal x matrix [128, 32] (bf16), rhs = strided
    fp8 bit-plane view [128, 512].  The {-1,+1} offset is fixed up by
    subtracting S_g = sum_{k in g} x_k (per-partition scalar) afterwards.
  * scale from alpha via ACT exp(ln2 * alpha); final contraction over (g,b)
    as PE transpose + DVE multiply-reduce; add bias; DMA out.
"""

import os
import sys

for _p in ("/opt/trn_rl_repo", "/opt/pypackages"):
    if os.path.isdir(_p) and _p not in sys.path:
        sys.path.insert(0, _p)

from contextlib import ExitStack

import ml_dtypes
import numpy as np

import concourse.bass as bass
import concourse.tile as tile
from concourse import bacc, mybir
from concourse._compat import with_exitstack
from concourse.bass_utils import run_bass_kernel_spmd

K = 4096
N = 4096
GROUP = 128
WBIT = 3
NUM_APOT = 3
G = K // GROUP          # 32 groups
NCORES = 8
NS = N // NCORES        # 512 output features per core
NBLK = NS // 128        # 4 partition-blocks of n' in the tail
WORDS = K // 32         # 128 packed words per (b, n)
Q = WBIT * G            # 96 (b, g) pairs
NA = NBLK * Q * NUM_APOT  # alpha cols: 1152
LN2 = float(np.log(2.0))

# packed fp32 consts tensor column layout
C_ID = 0                 # ident [96, 96]
C_AL = 96                # alphaf [128, 1152]
C_BI = C_AL + NA         # biasb [128, 4]
C_SG = C_BI + NBLK       # sg96 [96, 1]
C_TOT = C_SG + 1         # 1253

_CACHE = {}


@with_exitstack
def _build_kernel_body(ctx: ExitStack, tc):
    nc = tc.nc
    f32 = mybir.dt.float32
    i32 = mybir.dt.int32
    bf16 = mybir.dt.bfloat16
    f8 = mybir.dt.float8e4

    bw = nc.dram_tensor("bw", [WORDS, WBIT * NS], i32, kind="ExternalInput")
    xall = nc.dram_tensor("xall", [WORDS, G * G], bf16, kind="ExternalInput")
    consts = nc.dram_tensor("consts", [128, C_TOT], f32, kind="ExternalInput")
    out = nc.dram_tensor("out", [128, NBLK], f32, kind="ExternalOutput")

    sb = ctx.enter_context(tc.tile_pool(name="sb", bufs=1))
    psum = ctx.enter_context(tc.tile_pool(name="psum", bufs=1, space="PSUM"))
    psum_t = ctx.enter_context(tc.tile_pool(name="psum_t", bufs=1, space="PSUM"))

    # --- input DMAs (wsb first: it gates the whole pipeline) ------------------
    wsb = sb.tile([WORDS, WBIT * NS], i32)
    nc.sync.dma_start(wsb[:], bw[:, :])
    xall_sb = sb.tile([WORDS, G * G], bf16)
    nc.scalar.dma_start(xall_sb[:], xall[:, :])
    cst = sb.tile([128, C_TOT], f32)
    nc.scalar.dma_start(cst[:], consts[:, :])

    id_sb = cst[:Q, C_ID : C_ID + Q]
    al_sb = cst[:, C_AL : C_AL + NA]
    bias_sb = cst[:, C_BI : C_BI + NBLK]
    sg_sb = cst[:Q, C_SG : C_SG + 1]

    # --- scale[n', (blk, b, g)] = sum_a 2^alpha  (ACT exp + 2 DVE adds) -------
    scf = sb.tile([128, NA], f32)
    nc.scalar.activation(scf[:], al_sb, mybir.ActivationFunctionType.Exp, scale=LN2)
    scf3 = scf[:].rearrange("p (q a) -> p q a", a=NUM_APOT)
    sctmp = sb.tile([128, NBLK * Q], f32)
    nc.vector.tensor_tensor(sctmp[:], scf3[:, :, 0], scf3[:, :, 1], mybir.AluOpType.add)
    scale_t = sb.tile([128, NBLK * Q], f32)
    nc.vector.tensor_tensor(scale_t[:], sctmp[:], scf3[:, :, 2], mybir.AluOpType.add)

    # --- unpack 8 bit-in-byte planes (each op yields 4 byte-lane planes) ------
    planes = []
    for s in range(8):
        ps = sb.tile([WORDS, WBIT * NS], i32, tag=f"ps{s}", name=f"ps{s}")
        if s < 7:
            nc.vector.tensor_scalar(
                ps[:], wsb[:], 6 - s, 0x40404040,
                mybir.AluOpType.logical_shift_left, mybir.AluOpType.bitwise_and,
            )
        else:
            nc.vector.tensor_scalar(
                ps[:], wsb[:], 1, 0x40404040,
                mybir.AluOpType.logical_shift_right, mybir.AluOpType.bitwise_and,
            )
        planes.append(ps[:].bitcast(f8))  # [128, 4*WBIT*NS]

    # --- 96 accumulating matmuls: psum96[b*32+g, n'] = 2*sum_k x_k*bit --------
    psum96 = psum.tile([Q, NS], f32)
    for s in range(8):
        for c in range(4):
            j = 8 * c + s
            lhsT = xall_sb[:, j * G : (j + 1) * G]          # [128, 32] bf16
            for b in range(WBIT):
                base = 4 * (b * NS) + c
                rhs = planes[s][:, base : base + 4 * (NS - 1) + 1 : 4]  # [128,512]
                nc.tensor.matmul(
                    psum96[32 * b : 32 * b + 32, :],
                    lhsT,
                    rhs,
                    start=(s == 0 and c == 0),
                    stop=(s == 7 and c == 3),
                )

    # --- tail ----------------------------------------------------------------
    part96 = sb.tile([Q, NS], f32)
    nc.vector.tensor_scalar(
        part96[:], psum96[:], sg_sb, None, mybir.AluOpType.subtract
    )

    prod = sb.tile([128, NBLK * Q], f32)
    for blk in range(NBLK):
        pt = psum_t.tile([128, Q], f32, tag=f"pt{blk}", name=f"pt{blk}")
        nc.tensor.transpose(pt[:], part96[:, blk * 128 : (blk + 1) * 128], id_sb)
        nc.vector.tensor_tensor(
            prod[:, blk * Q : (blk + 1) * Q],
            pt[:],
            scale_t[:, blk * Q : (blk + 1) * Q],
            mybir.AluOpType.mult,
        )

    red = sb.tile([128, NBLK], f32)
    nc.vector.tensor_reduce(
        red[:],
        prod[:].rearrange("p (blk q) -> p blk q", blk=NBLK),
        mybir.AxisListType.X,
        mybir.AluOpType.add,
    )
    out_sb = sb.tile([128, NBLK], f32)
    nc.vector.tensor_tensor(out_sb[:], red[:], bias_sb, mybir.AluOpType.add)
    nc.sync.dma_start(out[:, :], out_sb[:])


def _get_nc():
    if "nc" not in _CACHE:
        nc = bacc.Bacc(
            "TRN2",
            target_bir_lowering=False,
            debug=False,
            enable_asserts=False,
            num_devices=NCORES,
        )
        with tile.TileContext(nc) as tc:
            _build_kernel_body(tc)
        nc.compile()
        _CACHE["nc"] = nc
    return _CACHE["nc"]


def _prep_inputs(x, binaryWeight, alpha, bias):
    """Host-side shard + layout prep (no arithmetic beyond tiny x-side sums)."""
    x = np.asarray(x, dtype=np.float32).reshape(K)
    binaryWeight = np.asarray(binaryWeight, dtype=np.int32)
    alpha = np.asarray(alpha, dtype=np.int32)
    bias = np.asarray(bias, dtype=np.float32).reshape(N)

    # Block-diagonal lhsT bank: xall[w, j*32 + g] = x[128g + 32(w-4g) + j]
    xall = np.zeros((WORDS, G, G), dtype=np.float32)  # [w, j, g]
    k = np.arange(K)
    g = k // GROUP
    sub = (k % GROUP) // 32
    j = k % 32
    xall[4 * g + sub, j, g] = x
    xall = xall.reshape(WORDS, G * G).astype(ml_dtypes.bfloat16)

    # S_g per group, tiled over b -> [96]
    sg = x.reshape(G, GROUP).sum(axis=1).astype(np.float32)

    in_maps = []
    for cc in range(NCORES):
        nsl = slice(cc * NS, (cc + 1) * NS)
        bw_sh = np.ascontiguousarray(binaryWeight[:, :, nsl]).reshape(
            WORDS, WBIT * NS
        )
        consts = np.zeros((128, C_TOT), dtype=np.float32)
        consts[:Q, C_ID : C_ID + Q] = np.eye(Q, dtype=np.float32)
        # alpha[n', b, g, a] -> [p, blk, b, g, a]  (n' = blk*128 + p)
        al = alpha[nsl].reshape(NBLK, 128, WBIT, G, NUM_APOT)
        al = al.transpose(1, 0, 2, 3, 4).reshape(128, NA)
        consts[:, C_AL : C_AL + NA] = al
        consts[:, C_BI : C_BI + NBLK] = bias[nsl].reshape(NBLK, 128).T
        consts[:Q, C_SG] = np.tile(sg, WBIT)
        in_maps.append({"bw": bw_sh, "xall": xall, "consts": consts})
    return in_maps


def _run(inputs, trace=False, **kw):
    nc = _get_nc()
    in_maps = _prep_inputs(**inputs)
    res = run_bass_kernel_spmd(
        nc, in_maps, core_ids=list(range(NCORES)), trace=trace, **kw
    )
    outs = []
    for cc in range(NCORES):
        o = res.results[cc]["out"]  # [128, NBLK]
        outs.append(np.ascontiguousarray(o.T).reshape(NS))  # n' = blk*128 + p
    full = np.concatenate(outs).reshape(1, N).astype(np.float32)
    return full, res


def kernel(**inputs):
    out, _ = _run(inputs, trace=False)
    return out


# revision 21
# speedup vs baseline: 1.6306x; 1.0120x over previous
"""Trainium2 Bass kernel for nn_LutLinear (BCQ/LUT-quantized linear layer).

Math (K=4096, N=4096, WBIT=3, GROUP=128, APOT=3):
  bits[k, b, n]  = bit (k%32) of binaryWeight[k//32, b, n]
  B              = 2*bits - 1                        (in {-1, +1})
  scale[n, b, g] = sum_a 2^alpha[n, b, g, a]
  out[n] = sum_{g,b} scale[n,b,g] * (sum_{k in group g} x[k] * B[k,b,n]) + bias[n]

Strategy (tensor-parallel over N, 8 cores, N'=512 each):
  * Bit-unpack on DVE: one int32 tensor_scalar (shift + AND 0x40404040) per
    bit-in-byte position s extracts FOUR bit-planes at once -- the masked
    int32, bitcast to fp8e4m3, holds value 2.0*bit in each of its 4 bytes.
  * PE computes partial[g,b,n'] = 2*sum_{k in g} x_k*bit via 96 accumulating
    matmuls: lhsT = block-diagonal x matrix [128, 32] (bf16), rhs = strided
    fp8 bit-plane view [128, 512].  The {-1,+1} offset is fixed up by
    subtracting S_g = sum_{k in g} x_k (per-partition scalar) afterwards.
  * scale from alpha via ACT exp(ln2 * alpha); final contraction over (g,b)
    as PE transpose + DVE multiply-reduce; add bias; DMA out.
"""

import os
import sys

for _p in ("/opt/trn_rl_repo", "/opt/pypackages"):
    if os.path.isdir(_p) and _p not in sys.path:
        sys.path.insert(0, _p)

from contextlib import ExitStack

import ml_dtypes
import numpy as np

import concourse.bass as bass
import concourse.tile as tile
from concourse import bacc, mybir
from concourse._compat import with_exitstack
from concourse.bass_utils import run_bass_kernel_spmd

K = 4096
N = 4096
GROUP = 128
WBIT = 3
NUM_APOT = 3
G = K // GROUP          # 32 groups
NCORES = 8
NS = N // NCORES        # 512 output features per core
NBLK = NS // 128        # 4 partition-blocks of n' in the tail
WORDS = K // 32         # 128 packed words per (b, n)
Q = WBIT * G            # 96 (b, g) pairs
NA = NBLK * Q * NUM_APOT  # alpha cols: 1152
LN2 = float(np.log(2.0))

# packed fp32 consts tensor column layout
C_ID = 0                 # ident [96, 96]
C_AL = 96                # alphaf [128, 1152]
C_BI = C_AL + NA         # biasb [128, 4]
C_SG = C_BI + NBLK       # sg96 [96, 1]
C_TOT = C_SG + 1         # 1253

_CACHE = {}


@with_exitstack
def _build_kernel_body(ctx: ExitStack, tc):
    nc = tc.nc
    f32 = mybir.dt.float32
    i32 = mybir.dt.int32
    bf16 = mybir.dt.bfloat16
    f8 = mybir.dt.float8e4

    bw = nc.dram_tensor("bw", [WORDS, WBIT * NS], i32, kind="ExternalInput")
    xall = nc.dram_tensor("xall", [WORDS, G * G], bf16, kind="ExternalInput")
    consts = nc.dram_tensor("consts", [128, C_TOT], f32, kind="ExternalInput")
    out = nc.dram_tensor("out", [128, NBLK], f32, kind="ExternalOutput")

    sb = ctx.enter_context(tc.tile_pool(name="sb", bufs=1))
    psum = ctx.enter_context(tc.tile_pool(name="psum", bufs=1, space="PSUM"))
    psum_t = ctx.enter_context(tc.tile_pool(name="psum_t", bufs=1, space="PSUM"))

    # --- input DMAs (weight slices first: they gate the pipeline) -------------
    bw3 = bw[:, :].rearrange("p (b n) -> p b n", b=WBIT)
    wsb = sb.tile([WORDS, WBIT * NS], i32)
    wsb3 = wsb[:].rearrange("p (b n) -> p b n", b=WBIT)
    nc.sync.dma_start(wsb3[:, 0, :], bw3[:, 0, :])
    nc.scalar.dma_start(wsb3[:, 1, :], bw3[:, 1, :])
    nc.gpsimd.dma_start(wsb3[:, 2, :], bw3[:, 2, :])
    xall_sb = sb.tile([WORDS, G * G], bf16)
    nc.scalar.dma_start(xall_sb[:], xall[:, :])
    cst = sb.tile([128, C_TOT], f32)
    nc.scalar.dma_start(cst[:], consts[:, :])

    # --- PE pre-warm: dummy matmuls on scratch data during the DMA wait.
    # HAM un-throttles the PE clock (1.2 -> 2.4 GHz) after ~3.4us of sustained
    # activity; burn that time while the weight DMAs are in flight so the real
    # matmul stream runs warm from its first instruction.
    warm = sb.tile([128, 544], bf16)
    nc.vector.memset(warm[:], 0.0)
    psum_w = psum.tile([32, NS], f32, tag="psum_w", name="psum_w")
    for _ in range(14):
        nc.tensor.matmul(
            psum_w[:, :], warm[:, :32], warm[:, 32:544], start=True, stop=True
        )

    id_sb = cst[:Q, C_ID : C_ID + Q]
    al_sb = cst[:, C_AL : C_AL + NA]
    bias_sb = cst[:, C_BI : C_BI + NBLK]
    sg_sb = cst[:Q, C_SG : C_SG + 1]

    # --- scale[n', (blk, b, g)] = sum_a 2^alpha  (ACT exp + GPSIMD adds) ------
    scf = sb.tile([128, NA], f32)
    nc.scalar.activation(scf[:], al_sb, mybir.ActivationFunctionType.Exp, scale=LN2)
    scf3 = scf[:].rearrange("p (q a) -> p q a", a=NUM_APOT)
    sctmp = sb.tile([128, NBLK * Q], f32)
    nc.vector.tensor_tensor(sctmp[:], scf3[:, :, 0], scf3[:, :, 1], mybir.AluOpType.add)
    scale_t = sb.tile([128, NBLK * Q], f32)
    nc.vector.tensor_tensor(scale_t[:], sctmp[:], scf3[:, :, 2], mybir.AluOpType.add)

    # --- unpack 8 bit-in-byte planes (each op yields 4 byte-lane planes) ------
    planes = []
    for s in range(8):
        ps = sb.tile([WORDS, WBIT * NS], i32, tag=f"ps{s}", name=f"ps{s}")
        if s < 7:
            nc.vector.tensor_scalar(
                ps[:], wsb[:], 6 - s, 0x40404040,
                mybir.AluOpType.logical_shift_left, mybir.AluOpType.bitwise_and,
            )
        else:
            nc.vector.tensor_scalar(
                ps[:], wsb[:], 1, 0x40404040,
                mybir.AluOpType.logical_shift_right, mybir.AluOpType.bitwise_and,
            )
        planes.append(ps[:].bitcast(f8))  # [128, 4*WBIT*NS]

    # --- 96 accumulating matmuls: psum96[b*32+g, n'] = 2*sum_k x_k*bit --------
    psum96 = psum.tile([Q, NS], f32)
    for s in range(8):
        for c in range(4):
            j = 8 * c + s
            lhsT = xall_sb[:, j * G : (j + 1) * G]          # [128, 32] bf16
            for b in range(WBIT):
                base = 4 * (b * NS) + c
                rhs = planes[s][:, base : base + 4 * (NS - 1) + 1 : 4]  # [128,512]
                nc.tensor.matmul(
                    psum96[32 * b : 32 * b + 32, :],
                    lhsT,
                    rhs,
                    start=(s == 0 and c == 0),
                    stop=(s == 7 and c == 3),
                )

    # --- tail ----------------------------------------------------------------
    part96 = sb.tile([Q, NS], f32)
    prod = sb.tile([128, NBLK * Q], f32)
    for blk in range(NBLK):
        nc.vector.tensor_scalar(
            part96[:, blk * 128 : (blk + 1) * 128],
            psum96[:, blk * 128 : (blk + 1) * 128],
            sg_sb, None, mybir.AluOpType.subtract,
        )
        pt = psum_t.tile([128, Q], f32, tag=f"pt{blk}", name=f"pt{blk}")
        nc.tensor.transpose(pt[:], part96[:, blk * 128 : (blk + 1) * 128], id_sb)
        nc.vector.tensor_tensor(
            prod[:, blk * Q : (blk + 1) * Q],
            pt[:],
            scale_t[:, blk * Q : (blk + 1) * Q],
            mybir.AluOpType.mult,
        )

    red = sb.tile([128, NBLK], f32)
    nc.vector.tensor_reduce(
        red[:],
        prod[:].rearrange("p (blk q) -> p blk q", blk=NBLK),
        mybir.AxisListType.X,
        mybir.AluOpType.add,
    )
    out_sb = sb.tile([128, NBLK], f32)
    nc.vector.tensor_tensor(out_sb[:], red[:], bias_sb, mybir.AluOpType.add)
    nc.sync.dma_start(out[:, :], out_sb[:])


def _get_nc():
    if "nc" not in _CACHE:
        nc = bacc.Bacc(
            "TRN2",
            target_bir_lowering=False,
            debug=False,
            enable_asserts=False,
            num_devices=1,
        )
        with tile.TileContext(nc) as tc:
            _build_kernel_body(tc)
        nc.compile()
        _CACHE["nc"] = nc
    return _CACHE["nc"]


def _prep_inputs(x, binaryWeight, alpha, bias):
    """Host-side shard + layout prep (no arithmetic beyond tiny x-side sums)."""
    x = np.asarray(x, dtype=np.float32).reshape(K)
    binaryWeight = np.asarray(binaryWeight, dtype=np.int32)
    alpha = np.asarray(alpha, dtype=np.int32)
    bias = np.asarray(bias, dtype=np.float32).reshape(N)

    # Block-diagonal lhsT bank: xall[w, j*32 + g] = x[128g + 32(w-4g) + j]
    xall = np.zeros((WORDS, G, G), dtype=np.float32)  # [w, j, g]
    k = np.arange(K)
    g = k // GROUP
    sub = (k % GROUP) // 32
    j = k % 32
    xall[4 * g + sub, j, g] = x
    xall = xall.reshape(WORDS, G * G).astype(ml_dtypes.bfloat16)

    # S_g per group, tiled over b -> [96]
    sg = x.reshape(G, GROUP).sum(axis=1).astype(np.float32)

    in_maps = []
    for cc in range(NCORES):
        nsl = slice(cc * NS, (cc + 1) * NS)
        bw_sh = np.ascontiguousarray(binaryWeight[:, :, nsl]).reshape(
            WORDS, WBIT * NS
        )
        consts = np.zeros((128, C_TOT), dtype=np.float32)
        consts[:Q, C_ID : C_ID + Q] = np.eye(Q, dtype=np.float32)
        # alpha[n', b, g, a] -> [p, blk, b, g, a]  (n' = blk*128 + p)
        al = alpha[nsl].reshape(NBLK, 128, WBIT, G, NUM_APOT)
        al = al.transpose(1, 0, 2, 3, 4).reshape(128, NA)
        consts[:, C_AL : C_AL + NA] = al
        consts[:, C_BI : C_BI + NBLK] = bias[nsl].reshape(NBLK, 128).T
        consts[:Q, C_SG] = np.tile(sg, WBIT)
        in_maps.append({"bw": bw_sh, "xall": xall, "consts": consts})
    return in_maps


def _run(inputs, trace=False, **kw):
    nc = _get_nc()
    in_maps = _prep_inputs(**inputs)
    res = run_bass_kernel_spmd(
        nc, in_maps, core_ids=list(range(NCORES)), trace=trace, **kw
    )
    outs = []
    for cc in range(NCORES):
        o = res.results[cc]["out"]  # [128, NBLK]
        outs.append(np.ascontiguousarray(o.T).reshape(NS))  # n' = blk*128 + p
    full = np.concatenate(outs).reshape(1, N).astype(np.float32)
    return full, res


def kernel(**inputs):
    out, _ = _run(inputs, trace=False)
    return out
